# revision 5
# baseline (speedup 1.0000x reference)
"""GATv2 (2-layer, 2-head) Trainium2 kernel, 8-core SPMD — lane-aligned v2.

Strategy: dst-node partition across 8 cores. Host assigns nodes to
(core, tile, lane) with a half-preserving two-pass packing: pass 1 sorts by
in-degree (fixes which gather half each node's slot is in), pass 2 re-sorts
within each half by (degA, degB) so tiles have uniform per-lane edge counts.
Edge slots are LANE-ALIGNED (slot partition == dst lane), so aggregation is
a plain identity-matmul accumulation over subtiles (512-wide PSUM pairs) and
xr[dst] is a per-tile broadcast — no per-edge xr gather, no mask build.
Scores: Prelu on ACT (column-sign handled by Prelu(u,.2)/Prelu(.2u,5)),
fp16 binary-tree folds on DVE, exp with -1024 bias masks invalid slots.
Full xl tables are built shard-wise and AllGathered in two chunks (A/B)
whose boundary doubles as the int16 gather-index split; layer-2 local
tables (loc2/own2) are fused into layer-1 finalize. Dense tail fused into
layer-2 finalize.
"""
import sys

sys.path.insert(0, "/opt/trn_rl_repo")

import numpy as np
import ml_dtypes

BF = ml_dtypes.bfloat16
F16 = np.float16

# ---- static layout constants ----
N = 50000
NCORES = 8
LANES = 128
NTILES = 49
SPC = NTILES * LANES          # 6272 slots per core
S = NCORES * SPC
NT_A = 25                     # tiles 0..24 -> gather half A
NT_B = 24
CHA = NT_A * LANES            # 3200
CHB = NT_B * LANES            # 3072
RA = NCORES * CHA             # 25600 rows in tabA (int16-safe)
RB = NCORES * CHB             # 24576
IN_F = 128
HC = 256
OUT_F = 40
SLOPE = 0.2
ROWB = 48                     # batch row budget (SBUF-bound)
SCB = 1024.0                  # score bias for invalid-slot masking

_NC_CACHE = {}
_RUN_OPTS = {}
_LAST_RESULTS = {}
_PLAN = {}


# ---------------------------------------------------------------- host prep
def _wrap_idx(idx):
    """[n] -> [128, n//16] wrapped (j at partition j%16, col j//16) + replicated."""
    n = idx.shape[0]
    a = idx.reshape(n // 16, 16).T.astype(np.int16)
    return np.tile(a, (8, 1))


def _pack_graph(src, dst):
    # peel one self-loop per node (all nodes have one: loops appended)
    is_self = src == dst
    self_eids = np.full(N, -1, np.int64)
    sids = np.where(is_self)[0]
    self_eids[src[sids]] = sids
    rand_mask = np.ones(len(src), bool)
    rand_mask[self_eids[self_eids >= 0]] = False
    rsrc, rdst = src[rand_mask], dst[rand_mask]

    deg = np.bincount(rdst, minlength=N)

    # pass 1: global deg sort, deal round-robin to cores, chunk -> halves
    order = np.argsort(-deg, kind="stable")
    core_of = np.empty(N, np.int32)
    core_of[order] = np.arange(N) % NCORES
    local1 = np.empty(N, np.int64)
    for k in range(NCORES):
        vs = order[core_of[order] == k]
        local1[vs] = np.arange(len(vs))
    halfA_node = local1 < CHA

    eA = halfA_node[rsrc]
    degA = np.bincount(rdst[eA], minlength=N)
    degB = deg - degA

    # pass 2: within each half sort by (degA, degB), chunk into that half's
    # tiles; lane = position within chunk
    tile_of_node = np.full(N, -1, np.int32)
    lane_of_node = np.full(N, -1, np.int32)
    for k in range(NCORES):
        vs = np.where(core_of == k)[0]
        A = vs[halfA_node[vs]]
        Bn = vs[~halfA_node[vs]]
        A = A[np.argsort(-(degA[A] * 1000 + degB[A]), kind="stable")]
        Bn = Bn[np.argsort(-(degA[Bn] * 1000 + degB[Bn]), kind="stable")]
        tile_of_node[A] = np.arange(len(A)) // LANES
        lane_of_node[A] = np.arange(len(A)) % LANES
        tile_of_node[Bn] = NT_A + np.arange(len(Bn)) // LANES
        lane_of_node[Bn] = np.arange(len(Bn)) % LANES

    slot_of_node = (core_of.astype(np.int64) * SPC
                    + tile_of_node.astype(np.int64) * LANES + lane_of_node)
    node_of_slot = np.full(S, -1, np.int64)
    node_of_slot[slot_of_node] = np.arange(N)

    # per-slot A/B counts -> common per-tile profile
    dstslot = slot_of_node[rdst]
    srcslot = slot_of_node[rsrc]
    loc_s = srcslot % SPC
    isB = loc_s >= CHA
    cntA = np.bincount(dstslot[~isB], minlength=S).reshape(NCORES, NTILES, 128)
    cntB = np.bincount(dstslot[isB], minlength=S).reshape(NCORES, NTILES, 128)
    TA = cntA.max(axis=(0, 2)).astype(np.int64)     # [NTILES] common
    TB = cntB.max(axis=(0, 2)).astype(np.int64)
    TBp = TB + (TA + TB + 1) % 2                    # ROWS even
    ROWS = TA + TBp + 1
    cumTA = np.concatenate([[0], np.cumsum(TA)])
    cumTB = np.concatenate([[0], np.cumsum(TBp)])
    row0 = np.concatenate([[0], np.cumsum(ROWS)])
    SUMA, SUMB, SUMR = int(cumTA[-1]), int(cumTB[-1]), int(row0[-1])

    # batches: consecutive tiles within row budget
    batches = []
    cur0, acc = 0, 0
    for t in range(NTILES):
        if acc + ROWS[t] > ROWB and acc > 0:
            batches.append((cur0, t - cur0))
            cur0, acc = t, 0
        acc += ROWS[t]
    batches.append((cur0, NTILES - cur0))

    # per-edge gather index + slot position
    ks = srcslot // SPC
    gidx = np.where(isB, ks * CHB + (loc_s - CHA),
                    ks * CHA + loc_s).astype(np.int64)
    kc = (dstslot // SPC).astype(np.int64)
    kt = ((dstslot % SPC) // LANES).astype(np.int64)
    kl = (dstslot % LANES).astype(np.int64)

    # rank within (dstslot, half)
    key = dstslot * 2 + isB
    es = np.argsort(key, kind="stable")
    gkey = key[es]
    start = np.ones(len(es), bool)
    start[1:] = gkey[1:] != gkey[:-1]
    gs = np.where(start, np.arange(len(es)), 0)
    rank = np.arange(len(es)) - np.maximum.accumulate(gs)

    e_kc = kc[es]
    e_kt = kt[es]
    e_kl = kl[es]
    e_isB = isB[es]
    e_gidx = gidx[es]

    idxA = np.zeros((NCORES, SUMA, 128), np.int16)
    idxB = np.zeros((NCORES, SUMB, 128), np.int16)
    valid = np.zeros((NCORES, SUMR, 128), F16)

    mA = ~e_isB
    idxA[e_kc[mA], cumTA[e_kt[mA]] + rank[mA], e_kl[mA]] = e_gidx[mA]
    valid[e_kc[mA], row0[e_kt[mA]] + rank[mA], e_kl[mA]] = 1.0
    mB = e_isB
    idxB[e_kc[mB], cumTB[e_kt[mB]] + rank[mB], e_kl[mB]] = e_gidx[mB]
    valid[e_kc[mB], row0[e_kt[mB]] + TA[e_kt[mB]] + rank[mB], e_kl[mB]] = 1.0
    # self rows (last row of each tile block), all real nodes
    valid[core_of, row0[tile_of_node] + ROWS[tile_of_node] - 1,
          lane_of_node] = 1.0

    # wrapped idx tensors [NCORES, 128, SUM*8]
    idxA_d = np.empty((NCORES, 128, SUMA * 8), np.int16)
    idxB_d = np.empty((NCORES, 128, SUMB * 8), np.int16)
    for k in range(NCORES):
        for t in range(NTILES):
            if TA[t]:
                idxA_d[k][:, cumTA[t] * 8:cumTA[t + 1] * 8] = _wrap_idx(
                    idxA[k, cumTA[t]:cumTA[t + 1]].reshape(-1))
            if TBp[t]:
                idxB_d[k][:, cumTB[t] * 8:cumTB[t + 1] * 8] = _wrap_idx(
                    idxB[k, cumTB[t]:cumTB[t + 1]].reshape(-1))
    valid_d = np.ascontiguousarray(valid.transpose(0, 2, 1))  # [NC,128,SUMR]

    _PLAN.update(TA=TA, TBp=TBp, ROWS=ROWS, cumTA=cumTA, cumTB=cumTB,
                 row0=row0, SUMA=SUMA, SUMB=SUMB, SUMR=SUMR, batches=batches)

    return dict(slot_of_node=slot_of_node, node_of_slot=node_of_slot,
                idxA_d=idxA_d, idxB_d=idxB_d, valid_d=valid_d)


# ---------------------------------------------------------------- device kernel
def _build_nc():
    import concourse.bass as bass
    import concourse.bacc as bacc
    import concourse.tile as tile
    import concourse.mybir as mybir

    F32 = mybir.dt.float32
    BF16 = mybir.dt.bfloat16
    FP16 = mybir.dt.float16
    I16 = mybir.dt.int16
    AF = mybir.ActivationFunctionType
    OP = mybir.AluOpType

    LR1, LR2 = _PLAN["LR1"], _PLAN["LR2"]
    TA, TBp, ROWS = _PLAN["TA"], _PLAN["TBp"], _PLAN["ROWS"]
    cumTA, cumTB, row0 = _PLAN["cumTA"], _PLAN["cumTB"], _PLAN["row0"]
    SUMA, SUMB, SUMR = _PLAN["SUMA"], _PLAN["SUMB"], _PLAN["SUMR"]
    batches = _PLAN["batches"]

    nc = bacc.Bacc(None, target_bir_lowering=False, num_swdge_queues=4)

    # ---- inputs
    xoT = nc.dram_tensor("xoT", [128, SPC], BF16, kind="ExternalInput")
    w1cat = nc.dram_tensor("w1cat", [128, 2 * HC], BF16, kind="ExternalInput")
    w2cat = nc.dram_tensor("w2cat", [128, 2, 2 * HC], BF16,
                           kind="ExternalInput")
    w3 = nc.dram_tensor("w3", [128, 2, 128], BF16, kind="ExternalInput")
    w4 = nc.dram_tensor("w4", [128, OUT_F], BF16, kind="ExternalInput")
    iav1 = nc.dram_tensor("iav1", [128, HC], F32, kind="ExternalInput")
    iav2 = nc.dram_tensor("iav2", [128, HC], F32, kind="ExternalInput")
    b1f = nc.dram_tensor("b1f", [128, HC], F32, kind="ExternalInput")
    b2f = nc.dram_tensor("b2f", [128, HC], F32, kind="ExternalInput")
    b3c = nc.dram_tensor("b3c", [128, 1], F32, kind="ExternalInput")
    b4f = nc.dram_tensor("b4f", [128, OUT_F], F32, kind="ExternalInput")
    idenBF = nc.dram_tensor("idenBF", [128, 128], BF16, kind="ExternalInput")
    idxA_d = nc.dram_tensor("idxA_d", [128, SUMA * 8], I16,
                            kind="ExternalInput")
    idxB_d = nc.dram_tensor("idxB_d", [128, SUMB * 8], I16,
                            kind="ExternalInput")
    valid_d = nc.dram_tensor("valid_d", [128, SUMR], FP16,
                             kind="ExternalInput")
    out_ext = nc.dram_tensor("out", [SPC, OUT_F], F32, kind="ExternalOutput")

    # ---- DRAM intermediates
    loc1 = nc.dram_tensor("loc1", [NTILES, 128, 2, HC], BF16)
    loc2 = nc.dram_tensor("loc2", [NTILES, 128, 2, HC], BF16)
    own1A = nc.dram_tensor("own1A", [NT_A, 128, HC], BF16)
    own1B = nc.dram_tensor("own1B", [NT_B, 128, HC], BF16)
    own2A = nc.dram_tensor("own2A", [NT_A, 128, HC], BF16)
    own2B = nc.dram_tensor("own2B", [NT_B, 128, HC], BF16)
    tab1A = nc.dram_tensor("tab1A", [NCORES, NT_A, 128, HC], BF16,
                           addr_space="Shared")
    tab1B = nc.dram_tensor("tab1B", [NCORES, NT_B, 128, HC], BF16,
                           addr_space="Shared")
    tab2A = nc.dram_tensor("tab2A", [NCORES, NT_A, 128, HC], BF16,
                           addr_space="Shared")
    tab2B = nc.dram_tensor("tab2B", [NCORES, NT_B, 128, HC], BF16,
                           addr_space="Shared")

    with tile.TileContext(nc) as tc:
        with (
            tc.tile_pool(name="const", bufs=1) as cpool,
            tc.tile_pool(name="tab", bufs=3) as tpool,
            tc.tile_pool(name="gath", bufs=2) as gpool,
            tc.tile_pool(name="work", bufs=2) as wpool,
            tc.tile_pool(name="fin", bufs=2) as fpool,
            tc.tile_pool(name="ups", bufs=2, space="PSUM") as psU,
            tc.tile_pool(name="psT", bufs=2, space="PSUM") as psT,
            tc.tile_pool(name="ps2", bufs=2, space="PSUM") as ps2p,
            tc.tile_pool(name="psF", bufs=1, space="PSUM") as psF,
        ):
            def load_const(t, shape, dt):
                tl = cpool.tile(shape, dt, tag=t.name)
                nc.sync.dma_start(out=tl[:], in_=t[:])
                return tl

            w1_sb = load_const(w1cat, [128, 2 * HC], BF16)
            w2_sb = load_const(w2cat, [128, 2, 2 * HC], BF16)
            w3_sb = load_const(w3, [128, 2, 128], BF16)
            w4_sb = load_const(w4, [128, OUT_F], BF16)
            iav1_sb = load_const(iav1, [128, HC], F32)
            iav2_sb = load_const(iav2, [128, HC], F32)
            b1f_sb = load_const(b1f, [128, HC], F32)
            b2f_sb = load_const(b2f, [128, HC], F32)
            b3c_sb = load_const(b3c, [128, 1], F32)
            b4f_sb = load_const(b4f, [128, OUT_F], F32)
            iden_sb = load_const(idenBF, [128, 128], BF16)
            xoT_sb = load_const(xoT, [128, SPC], BF16)
            nbias = cpool.tile([128, 1], F32, tag="nbias")
            nc.vector.memset(nbias[:], -SCB)

            # ---------- L1 tables: per tile [xl1|xr1], own chunks for AllGather
            for t in range(NTILES):
                ps = ps2p.tile([128, 2 * HC], F32, tag="ps2")
                nc.tensor.matmul(ps[:], xoT_sb[:, t * 128:(t + 1) * 128],
                                 w1_sb[:], start=True, stop=True)
                lt = tpool.tile([128, 2, HC], BF16, tag="lt")
                nc.vector.tensor_copy(lt[:, 0, :], ps[:, 0:HC])
                nc.scalar.activation(lt[:, 1, :], ps[:, HC:2 * HC], AF.Copy)
                nc.sync.dma_start(out=loc1[t], in_=lt[:])
                if t < NT_A:
                    nc.scalar.dma_start(out=own1A[t], in_=lt[:, 0, :])
                else:
                    nc.scalar.dma_start(out=own1B[t - NT_A], in_=lt[:, 0, :])

            nc.gpsimd.collective_compute(
                "AllGather", mybir.AluOpType.bypass,
                replica_groups=[list(range(NCORES))],
                ins=[own1A.ap().opt()], outs=[tab1A.ap().opt()])
            nc.gpsimd.collective_compute(
                "AllGather", mybir.AluOpType.bypass,
                replica_groups=[list(range(NCORES))],
                ins=[own1B.ap().opt()], outs=[tab1B.ap().opt()])

            # ---------- conv layer
            qctr = [0]

            def conv(tabA, tabB, loc_tab, lrr, iav_sb, bf_sb, fin):
                tabAf = tabA.rearrange("k t p c -> (k t p) c")
                tabBf = tabB.rearrange("k t p c -> (k t p) c")
                for (t0, nb) in batches:
                    r0 = int(row0[t0])
                    rows = int(row0[t0 + nb] - r0)
                    a0, aN = int(cumTA[t0]), int(cumTA[t0 + nb] - cumTA[t0])
                    b0, bN = int(cumTB[t0]), int(cumTB[t0 + nb] - cumTB[t0])
                    ixa = gpool.tile([128, max(aN, 1) * 8], I16, tag="ixa")
                    if aN:
                        nc.sync.dma_start(
                            out=ixa[:], in_=idxA_d[:, a0 * 8:(a0 + aN) * 8])
                    ixb = gpool.tile([128, max(bN, 1) * 8], I16, tag="ixb")
                    if bN:
                        nc.sync.dma_start(
                            out=ixb[:], in_=idxB_d[:, b0 * 8:(b0 + bN) * 8])
                    vd = gpool.tile([128, rows], FP16, tag="vd")
                    nc.sync.dma_start(out=vd[:], in_=valid_d[:, r0:r0 + rows])
                    sxr = gpool.tile([128, nb, HC], BF16, tag="sxr")
                    nc.sync.dma_start(
                        out=sxr[:],
                        in_=loc_tab[t0:t0 + nb, :, 1, :].rearrange(
                            "t p c -> p t c"))
                    g = gpool.tile([128, rows, HC], BF16, tag="g")
                    for n in range(nb):
                        t = t0 + n
                        ta, tb = int(TA[t]), int(TBp[t])
                        ro = int(row0[t]) - r0
                        if ta:
                            nc.gpsimd.dma_gather(
                                out_ap=g[:, ro:ro + ta, :], in_ap=tabAf[:],
                                idxs_ap=ixa[:, (int(cumTA[t]) - a0) * 8:
                                            (int(cumTA[t + 1]) - a0) * 8],
                                num_idxs=ta * 128, num_idxs_reg=ta * 128,
                                elem_size=HC, single_packet=False,
                                queue_num=qctr[0] % 4)
                            qctr[0] += 1
                        if tb:
                            nc.gpsimd.dma_gather(
                                out_ap=g[:, ro + ta:ro + ta + tb, :],
                                in_ap=tabBf[:],
                                idxs_ap=ixb[:, (int(cumTB[t]) - b0) * 8:
                                            (int(cumTB[t + 1]) - b0) * 8],
                                num_idxs=tb * 128, num_idxs_reg=tb * 128,
                                elem_size=HC, single_packet=False,
                                queue_num=qctr[0] % 4)
                            qctr[0] += 1
                        # self xl row (last row of tile block)
                        nc.scalar.dma_start(
                            out=g[:, ro + int(ROWS[t]) - 1, :],
                            in_=loc_tab[t, :, 0, :])
                    # ub = g + xr (per tile broadcast), fp16
                    ub = wpool.tile([128, rows, HC], FP16, tag="ub")
                    for n in range(nb):
                        t = t0 + n
                        ro = int(row0[t]) - r0
                        rt = int(ROWS[t])
                        nc.vector.tensor_tensor(
                            out=ub[:, ro:ro + rt, :], in0=g[:, ro:ro + rt, :],
                            in1=sxr[:, n, :].rearrange(
                                "p c -> p () c").broadcast_to([128, rt, HC]),
                            op=OP.add)
                    # leaky-relu on ACT via Prelu (sign-folded ranges)
                    for (c0, c1, kind) in lrr:
                        if kind == "max":
                            nc.scalar.activation(
                                ub[:, :, c0:c1], ub[:, :, c0:c1], AF.Prelu,
                                alpha=SLOPE)
                        else:
                            nc.scalar.activation(
                                ub[:, :, c0:c1], ub[:, :, c0:c1], AF.Prelu,
                                scale=SLOPE, alpha=1.0 / SLOPE)
                    # fold c 128->1 per head (fp16 tree)
                    ubv = ub[:].rearrange("p r (h c) -> p r h c", h=2)
                    for w in (64, 32, 16, 8, 4, 2):
                        nc.vector.tensor_tensor(
                            out=ubv[:, :, :, 0:w], in0=ubv[:, :, :, 0:w],
                            in1=ubv[:, :, :, w:2 * w], op=OP.add)
                    sc = wpool.tile([128, rows, 2], F32, tag="sc")
                    nc.vector.tensor_tensor(
                        out=sc[:].rearrange("p r h -> p r h ()"),
                        in0=ubv[:, :, :, 0:1], in1=ubv[:, :, :, 1:2],
                        op=OP.add)
                    # mask invalid slots then exp
                    nc.vector.scalar_tensor_tensor(
                        out=sc[:], in0=sc[:], scalar=SCB,
                        in1=vd[:].rearrange("p r -> p r ()").broadcast_to(
                            [128, rows, 2]),
                        op0=OP.add, op1=OP.mult)
                    afb = wpool.tile([128, rows, 2], BF16, tag="afb")
                    nc.scalar.activation(afb[:], sc[:], AF.Exp, bias=nbias[:])
                    # denominators per tile
                    den = wpool.tile([128, nb, 2], F32, tag="den")
                    for n in range(nb):
                        t = t0 + n
                        ro = int(row0[t]) - r0
                        nc.vector.tensor_reduce(
                            out=den[:, n, :].rearrange("p h -> p h ()"),
                            in_=afb[:, ro:ro + int(ROWS[t]), :].rearrange(
                                "p t h -> p h t"),
                            axis=mybir.AxisListType.X, op=OP.add)
                    nc.vector.tensor_scalar(
                        out=den[:], in0=den[:], scalar1=1e-16, scalar2=None,
                        op0=OP.add)
                    rden = wpool.tile([128, nb, 2], F32, tag="rden")
                    nc.vector.reciprocal(rden[:], den[:])
                    # ya = alpha * xl
                    ya = wpool.tile([128, rows, HC], BF16, tag="ya")
                    nc.vector.tensor_tensor(
                        out=ya[:].rearrange("p r (h c) -> p r h c", h=2),
                        in0=g[:].rearrange("p r (h c) -> p r h c", h=2),
                        in1=afb[:].rearrange("p r h -> p r h ()").broadcast_to(
                            [128, rows, 2, 128]),
                        op=OP.mult)
                    # aggregate per tile: identity matmuls, 512-wide pairs
                    yf = ya[:].rearrange("p r c -> p (r c)")
                    for n in range(nb):
                        t = t0 + n
                        ro = int(row0[t]) - r0
                        npair = int(ROWS[t]) // 2
                        ups = psU.tile([128, 512], F32, tag="ups")
                        for j in range(npair):
                            o = (ro + 2 * j) * HC
                            nc.tensor.matmul(
                                ups[:], iden_sb[:], yf[:, o:o + 512],
                                start=(j == 0), stop=(j == npair - 1))
                        fin(t, ups, rden, n, iav_sb, bf_sb)

            # ---------- finalize helpers
            def head_merge(ups, rden, n, iav_sb, bf_sb):
                upB = fpool.tile([128, HC], F32, tag="upB")
                nc.scalar.activation(upB[:], ups[:, HC:2 * HC], AF.Copy)
                u = fpool.tile([128, HC], F32, tag="u")
                nc.vector.tensor_tensor(
                    out=u[:], in0=ups[:, 0:HC], in1=upB[:], op=OP.add)
                for h in range(2):
                    nc.vector.tensor_scalar(
                        out=u[:, h * 128:(h + 1) * 128],
                        in0=u[:, h * 128:(h + 1) * 128],
                        scalar1=rden[:, n, h:h + 1], scalar2=None,
                        op0=OP.mult)
                nc.vector.tensor_tensor(out=u[:], in0=u[:], in1=iav_sb[:],
                                        op=OP.mult)
                nc.vector.tensor_tensor(out=u[:], in0=u[:], in1=bf_sb[:],
                                        op=OP.add)
                hbf = fpool.tile([128, HC], BF16, tag="hbf")
                nc.scalar.activation(hbf[:], u[:], AF.Relu)
                ct = fpool.tile([128, 2, 128], BF16, tag="ct")
                for h in range(2):
                    pt = psT.tile([128, 128], BF16, tag="pt")
                    nc.tensor.transpose(pt[:], hbf[:, h * 128:(h + 1) * 128],
                                        iden_sb[:])
                    if h == 0:
                        nc.scalar.activation(ct[:, h, :], pt[:], AF.Copy)
                    else:
                        nc.vector.tensor_copy(ct[:, h, :], pt[:])
                return ct

            def fin1(t, ups, rden, n, iav_sb, bf_sb):
                ct = head_merge(ups, rden, n, iav_sb, bf_sb)
                ps2 = ps2p.tile([128, 2 * HC], F32, tag="ps2")
                nc.tensor.matmul(ps2[:], ct[:, 0, :], w2_sb[:, 0, :],
                                 start=True, stop=False)
                nc.tensor.matmul(ps2[:], ct[:, 1, :], w2_sb[:, 1, :],
                                 start=False, stop=True)
                l2t = fpool.tile([128, 2, HC], BF16, tag="l2t")
                nc.vector.tensor_copy(l2t[:, 0, :], ps2[:, 0:HC])
                nc.scalar.activation(l2t[:, 1, :], ps2[:, HC:2 * HC], AF.Copy)
                nc.sync.dma_start(out=loc2[t], in_=l2t[:])
                if t < NT_A:
                    nc.scalar.dma_start(out=own2A[t], in_=l2t[:, 0, :])
                else:
                    nc.scalar.dma_start(out=own2B[t - NT_A], in_=l2t[:, 0, :])

            def fin2(t, ups, rden, n, iav_sb, bf_sb):
                ct = head_merge(ups, rden, n, iav_sb, bf_sb)
                zt_ps = psF.tile([128, 128], F32, tag="ztps")
                nc.tensor.matmul(zt_ps[:], w3_sb[:, 0, :], ct[:, 0, :],
                                 start=True, stop=False)
                nc.tensor.matmul(zt_ps[:], w3_sb[:, 1, :], ct[:, 1, :],
                                 start=False, stop=True)
                zt = fpool.tile([128, 128], BF16, tag="zt")
                nc.scalar.activation(zt[:], zt_ps[:], AF.Identity,
                                     bias=b3c_sb[:], scale=1.0)
                o_ps = psF.tile([128, OUT_F], F32, tag="ops")
                nc.tensor.matmul(o_ps[:], zt[:], w4_sb[:], start=True,
                                 stop=True)
                o_pre = fpool.tile([128, OUT_F], F32, tag="opre")
                nc.vector.scalar_tensor_tensor(
                    out=o_pre[:], in0=o_ps[:], scalar=1.0, in1=b4f_sb[:],
                    op0=OP.mult, op1=OP.add)
                o_sb = fpool.tile([128, OUT_F], F32, tag="osb")
                nc.scalar.activation(o_sb[:], o_pre[:], AF.Sigmoid)
                nc.sync.dma_start(out=out_ext[t * 128:(t + 1) * 128, :],
                                  in_=o_sb[:])

            # ================= phase schedule =================
            import os as _os
            _upto = int(_os.environ.get("KPHASES", "9"))
            if _upto >= 2:
                conv(tab1A, tab1B, loc1, LR1, iav1_sb, b1f_sb, fin1)
            if _upto >= 3:
                nc.gpsimd.collective_compute(
                    "AllGather", mybir.AluOpType.bypass,
                    replica_groups=[list(range(NCORES))],
                    ins=[own2A.ap().opt()], outs=[tab2A.ap().opt()])
                nc.gpsimd.collective_compute(
                    "AllGather", mybir.AluOpType.bypass,
                    replica_groups=[list(range(NCORES))],
                    ins=[own2B.ap().opt()], outs=[tab2B.ap().opt()])
            if _upto >= 4:
                conv(tab2A, tab2B, loc2, LR2, iav2_sb, b2f_sb, fin2)
            else:
                zt = fpool.tile([128, OUT_F], F32, tag="osb")
                nc.vector.memset(zt[:], 0.0)
                for t in range(NTILES):
                    nc.sync.dma_start(out=out_ext[t * 128:(t + 1) * 128, :],
                                      in_=zt[:])

    nc.compile()
    return nc


# ---------------------------------------------------------------- entry point
def kernel(**inputs):
    from concourse import bass_utils

    src = np.asarray(inputs["edge_index"][0], np.int64)
    dst = np.asarray(inputs["edge_index"][1], np.int64)
    x = np.asarray(inputs["x"], np.float32)

    pack = _pack_graph(src, dst)
    nos = pack["node_of_slot"]
    valid_slot = nos >= 0
    x_slot = np.zeros((S, IN_F), np.float32)
    x_slot[valid_slot] = x[nos[valid_slot]]

    def bf(a):
        return np.ascontiguousarray(np.asarray(a, np.float32)).astype(BF)

    # per-head column permutation (+att cols first) + pre-scale by att
    def prep_layer(att):
        att = np.asarray(att, np.float32).reshape(2, 128)
        perm = np.zeros(HC, np.int64)
        ranges = []
        for h in range(2):
            a = att[h]
            pos = np.where(a > 0)[0]
            neg = np.where(a <= 0)[0]
            perm[h * 128:(h + 1) * 128] = h * 128 + np.concatenate([pos, neg])
            p = len(pos)
            if p:
                ranges.append((h * 128, h * 128 + p, "max"))
            if p < 128:
                ranges.append((h * 128 + p, (h + 1) * 128, "min"))
        att_p = att.reshape(HC)[perm]
        att_p = np.where(np.abs(att_p) < 1e-30, 1e-30, att_p)
        return perm, att_p, ranges

    perm1, att1p, LR1 = prep_layer(inputs["att1"])
    perm2, att2p, LR2 = prep_layer(inputs["att2"])
    _PLAN["LR1"] = LR1
    _PLAN["LR2"] = LR2

    Wl1p = np.asarray(inputs["Wl1"], np.float32)[:, perm1] * att1p[None, :]
    Wr1p = np.asarray(inputs["Wr1"], np.float32)[:, perm1] * att1p[None, :]
    Wl2p = (np.asarray(inputs["Wl2"], np.float32)[perm1][:, perm2]
            * att2p[None, :])
    Wr2p = (np.asarray(inputs["Wr2"], np.float32)[perm1][:, perm2]
            * att2p[None, :])
    W3p = np.asarray(inputs["W3"], np.float32)[perm2]
    b1p = np.asarray(inputs["b1"], np.float32)[perm1]
    b2p = np.asarray(inputs["b2"], np.float32)[perm2]

    w2c = np.concatenate([Wl2p, Wr2p], 1)           # [256, 512]
    common = {
        "w1cat": bf(np.concatenate([Wl1p, Wr1p], 1)),
        "w2cat": bf(w2c.reshape(2, 128, 2 * HC).transpose(1, 0, 2)),
        "w3": bf(W3p.reshape(2, 128, 128).transpose(1, 0, 2)),
        "w4": bf(inputs["W4"]),
        "iav1": np.tile((1.0 / att1p)[None, :], (128, 1)).astype(np.float32),
        "iav2": np.tile((1.0 / att2p)[None, :], (128, 1)).astype(np.float32),
        "b1f": np.tile(b1p[None, :], (128, 1)),
        "b2f": np.tile(b2p[None, :], (128, 1)),
        "b3c": np.asarray(inputs["b3"], np.float32).reshape(128, 1),
        "b4f": np.tile(np.asarray(inputs["b4"], np.float32)[None, :],
                       (128, 1)),
        "idenBF": np.eye(128, dtype=np.float32).astype(BF),
    }

    in_maps = []
    for k in range(NCORES):
        m = dict(common)
        m["xoT"] = np.ascontiguousarray(
            x_slot[k * SPC:(k + 1) * SPC].T).astype(BF)
        m["idxA_d"] = pack["idxA_d"][k]
        m["idxB_d"] = pack["idxB_d"][k]
        m["valid_d"] = pack["valid_d"][k]
        in_maps.append(m)

    if "nc" not in _NC_CACHE:
        _NC_CACHE["nc"] = _build_nc()
    nc = _NC_CACHE["nc"]

    res = bass_utils.run_bass_kernel_spmd(nc, in_maps,
                                          core_ids=list(range(NCORES)),
                                          **_RUN_OPTS)
    _LAST_RESULTS["res"] = res
    out_slots = np.concatenate([res.results[k]["out"] for k in range(NCORES)],
                               0)
    return out_slots[pack["slot_of_node"]].astype(np.float32)


# revision 7
# speedup vs baseline: 1.0272x; 1.0272x over previous
"""GATv2 (2-layer, 2-head) Trainium2 kernel, 8-core SPMD — lane-aligned v2.

Strategy: dst-node partition across 8 cores. Host assigns nodes to
(core, tile, lane) with a half-preserving two-pass packing: pass 1 sorts by
in-degree (fixes which gather half each node's slot is in), pass 2 re-sorts
within each half by (degA, degB) so tiles have uniform per-lane edge counts.
Edge slots are LANE-ALIGNED (slot partition == dst lane), so aggregation is
a plain identity-matmul accumulation over subtiles (512-wide PSUM pairs) and
xr[dst] is a per-tile broadcast — no per-edge xr gather, no mask build.
Scores: Prelu on ACT (column-sign handled by Prelu(u,.2)/Prelu(.2u,5)),
fp16 binary-tree folds on DVE, exp with -1024 bias masks invalid slots.
Full xl tables are built shard-wise and AllGathered in two chunks (A/B)
whose boundary doubles as the int16 gather-index split; layer-2 local
tables (loc2/own2) are fused into layer-1 finalize. Dense tail fused into
layer-2 finalize.
"""
import sys

sys.path.insert(0, "/opt/trn_rl_repo")

import numpy as np
import ml_dtypes

BF = ml_dtypes.bfloat16
F16 = np.float16

# ---- static layout constants ----
N = 50000
NCORES = 8
LANES = 128
NTILES = 49
SPC = NTILES * LANES          # 6272 slots per core
S = NCORES * SPC
NT_A = 25                     # tiles 0..24 -> gather half A
NT_B = 24
CHA = NT_A * LANES            # 3200
CHB = NT_B * LANES            # 3072
RA = NCORES * CHA             # 25600 rows in tabA (int16-safe)
RB = NCORES * CHB             # 24576
IN_F = 128
HC = 256
OUT_F = 40
SLOPE = 0.2
ROWB = 48                     # batch row budget (SBUF-bound)
SCB = 1024.0                  # score bias for invalid-slot masking

_NC_CACHE = {}
_RUN_OPTS = {}
_LAST_RESULTS = {}
_PLAN = {}


# ---------------------------------------------------------------- host prep
def _wrap_idx(idx):
    """[n] -> [128, n//16] wrapped (j at partition j%16, col j//16) + replicated."""
    n = idx.shape[0]
    a = idx.reshape(n // 16, 16).T.astype(np.int16)
    return np.tile(a, (8, 1))


def _pack_graph(src, dst):
    # peel one self-loop per node (all nodes have one: loops appended)
    is_self = src == dst
    self_eids = np.full(N, -1, np.int64)
    sids = np.where(is_self)[0]
    self_eids[src[sids]] = sids
    rand_mask = np.ones(len(src), bool)
    rand_mask[self_eids[self_eids >= 0]] = False
    rsrc, rdst = src[rand_mask], dst[rand_mask]

    deg = np.bincount(rdst, minlength=N)

    # pass 1: global deg sort, deal round-robin to cores, chunk -> halves
    order = np.argsort(-deg, kind="stable")
    core_of = np.empty(N, np.int32)
    core_of[order] = np.arange(N) % NCORES
    local1 = np.empty(N, np.int64)
    for k in range(NCORES):
        vs = order[core_of[order] == k]
        local1[vs] = np.arange(len(vs))
    halfA_node = local1 < CHA

    eA = halfA_node[rsrc]
    degA = np.bincount(rdst[eA], minlength=N)
    degB = deg - degA

    # pass 2: within each half sort by (degA, degB), chunk into that half's
    # tiles; lane = position within chunk
    tile_of_node = np.full(N, -1, np.int32)
    lane_of_node = np.full(N, -1, np.int32)
    for k in range(NCORES):
        vs = np.where(core_of == k)[0]
        A = vs[halfA_node[vs]]
        Bn = vs[~halfA_node[vs]]
        A = A[np.argsort(-(degA[A] * 1000 + degB[A]), kind="stable")]
        Bn = Bn[np.argsort(-(degA[Bn] * 1000 + degB[Bn]), kind="stable")]
        tile_of_node[A] = np.arange(len(A)) // LANES
        lane_of_node[A] = np.arange(len(A)) % LANES
        tile_of_node[Bn] = NT_A + np.arange(len(Bn)) // LANES
        lane_of_node[Bn] = np.arange(len(Bn)) % LANES

    slot_of_node = (core_of.astype(np.int64) * SPC
                    + tile_of_node.astype(np.int64) * LANES + lane_of_node)
    node_of_slot = np.full(S, -1, np.int64)
    node_of_slot[slot_of_node] = np.arange(N)

    # per-slot A/B counts -> common per-tile profile
    dstslot = slot_of_node[rdst]
    srcslot = slot_of_node[rsrc]
    loc_s = srcslot % SPC
    isB = loc_s >= CHA
    cntA = np.bincount(dstslot[~isB], minlength=S).reshape(NCORES, NTILES, 128)
    cntB = np.bincount(dstslot[isB], minlength=S).reshape(NCORES, NTILES, 128)
    TA = cntA.max(axis=(0, 2)).astype(np.int64)     # [NTILES] common
    TB = cntB.max(axis=(0, 2)).astype(np.int64)
    TBp = TB + (TA + TB + 1) % 2                    # ROWS even
    ROWS = TA + TBp + 1
    cumTA = np.concatenate([[0], np.cumsum(TA)])
    cumTB = np.concatenate([[0], np.cumsum(TBp)])
    row0 = np.concatenate([[0], np.cumsum(ROWS)])
    SUMA, SUMB, SUMR = int(cumTA[-1]), int(cumTB[-1]), int(row0[-1])

    # batches: consecutive tiles within row budget
    batches = []
    cur0, acc = 0, 0
    for t in range(NTILES):
        if acc + ROWS[t] > ROWB and acc > 0:
            batches.append((cur0, t - cur0))
            cur0, acc = t, 0
        acc += ROWS[t]
    batches.append((cur0, NTILES - cur0))

    # per-edge gather index + slot position
    ks = srcslot // SPC
    gidx = np.where(isB, ks * CHB + (loc_s - CHA),
                    ks * CHA + loc_s).astype(np.int64)
    kc = (dstslot // SPC).astype(np.int64)
    kt = ((dstslot % SPC) // LANES).astype(np.int64)
    kl = (dstslot % LANES).astype(np.int64)

    # rank within (dstslot, half)
    key = dstslot * 2 + isB
    es = np.argsort(key, kind="stable")
    gkey = key[es]
    start = np.ones(len(es), bool)
    start[1:] = gkey[1:] != gkey[:-1]
    gs = np.where(start, np.arange(len(es)), 0)
    rank = np.arange(len(es)) - np.maximum.accumulate(gs)

    e_kc = kc[es]
    e_kt = kt[es]
    e_kl = kl[es]
    e_isB = isB[es]
    e_gidx = gidx[es]

    idxA = np.zeros((NCORES, SUMA, 128), np.int16)
    idxB = np.zeros((NCORES, SUMB, 128), np.int16)
    valid = np.zeros((NCORES, SUMR, 128), F16)

    mA = ~e_isB
    idxA[e_kc[mA], cumTA[e_kt[mA]] + rank[mA], e_kl[mA]] = e_gidx[mA]
    valid[e_kc[mA], row0[e_kt[mA]] + rank[mA], e_kl[mA]] = 1.0
    mB = e_isB
    idxB[e_kc[mB], cumTB[e_kt[mB]] + rank[mB], e_kl[mB]] = e_gidx[mB]
    valid[e_kc[mB], row0[e_kt[mB]] + TA[e_kt[mB]] + rank[mB], e_kl[mB]] = 1.0
    # self rows (last row of each tile block), all real nodes
    valid[core_of, row0[tile_of_node] + ROWS[tile_of_node] - 1,
          lane_of_node] = 1.0

    # wrapped idx tensors [NCORES, 128, SUM*8]
    idxA_d = np.empty((NCORES, 128, SUMA * 8), np.int16)
    idxB_d = np.empty((NCORES, 128, SUMB * 8), np.int16)
    for k in range(NCORES):
        for t in range(NTILES):
            if TA[t]:
                idxA_d[k][:, cumTA[t] * 8:cumTA[t + 1] * 8] = _wrap_idx(
                    idxA[k, cumTA[t]:cumTA[t + 1]].reshape(-1))
            if TBp[t]:
                idxB_d[k][:, cumTB[t] * 8:cumTB[t + 1] * 8] = _wrap_idx(
                    idxB[k, cumTB[t]:cumTB[t + 1]].reshape(-1))
    valid_d = np.ascontiguousarray(valid.transpose(0, 2, 1))  # [NC,128,SUMR]

    _PLAN.update(TA=TA, TBp=TBp, ROWS=ROWS, cumTA=cumTA, cumTB=cumTB,
                 row0=row0, SUMA=SUMA, SUMB=SUMB, SUMR=SUMR, batches=batches)

    return dict(slot_of_node=slot_of_node, node_of_slot=node_of_slot,
                idxA_d=idxA_d, idxB_d=idxB_d, valid_d=valid_d)


# ---------------------------------------------------------------- device kernel
def _build_nc():
    import concourse.bass as bass
    import concourse.bacc as bacc
    import concourse.tile as tile
    import concourse.mybir as mybir

    F32 = mybir.dt.float32
    BF16 = mybir.dt.bfloat16
    FP16 = mybir.dt.float16
    I16 = mybir.dt.int16
    AF = mybir.ActivationFunctionType
    OP = mybir.AluOpType

    LR1, LR2 = _PLAN["LR1"], _PLAN["LR2"]
    TA, TBp, ROWS = _PLAN["TA"], _PLAN["TBp"], _PLAN["ROWS"]
    cumTA, cumTB, row0 = _PLAN["cumTA"], _PLAN["cumTB"], _PLAN["row0"]
    SUMA, SUMB, SUMR = _PLAN["SUMA"], _PLAN["SUMB"], _PLAN["SUMR"]
    batches = _PLAN["batches"]

    nc = bacc.Bacc(None, target_bir_lowering=False, num_swdge_queues=4)

    # ---- inputs
    xoT = nc.dram_tensor("xoT", [128, SPC], BF16, kind="ExternalInput")
    w1cat = nc.dram_tensor("w1cat", [128, 2 * HC], BF16, kind="ExternalInput")
    w2cat = nc.dram_tensor("w2cat", [128, 2, 2 * HC], BF16,
                           kind="ExternalInput")
    w3 = nc.dram_tensor("w3", [128, 2, 128], BF16, kind="ExternalInput")
    w4 = nc.dram_tensor("w4", [128, OUT_F], BF16, kind="ExternalInput")
    iav1 = nc.dram_tensor("iav1", [128, HC], F32, kind="ExternalInput")
    iav2 = nc.dram_tensor("iav2", [128, HC], F32, kind="ExternalInput")
    b1f = nc.dram_tensor("b1f", [128, HC], F32, kind="ExternalInput")
    b2f = nc.dram_tensor("b2f", [128, HC], F32, kind="ExternalInput")
    b3c = nc.dram_tensor("b3c", [128, 1], F32, kind="ExternalInput")
    b4f = nc.dram_tensor("b4f", [128, OUT_F], F32, kind="ExternalInput")
    idenBF = nc.dram_tensor("idenBF", [128, 128], BF16, kind="ExternalInput")
    idxA_d = nc.dram_tensor("idxA_d", [128, SUMA * 8], I16,
                            kind="ExternalInput")
    idxB_d = nc.dram_tensor("idxB_d", [128, SUMB * 8], I16,
                            kind="ExternalInput")
    valid_d = nc.dram_tensor("valid_d", [128, SUMR], FP16,
                             kind="ExternalInput")
    out_ext = nc.dram_tensor("out", [SPC, OUT_F], F32, kind="ExternalOutput")

    # ---- DRAM intermediates
    loc1 = nc.dram_tensor("loc1", [NTILES, 128, 2, HC], BF16)
    loc2 = nc.dram_tensor("loc2", [NTILES, 128, 2, HC], BF16)
    own1A = nc.dram_tensor("own1A", [NT_A, 128, HC], BF16)
    own1B = nc.dram_tensor("own1B", [NT_B, 128, HC], BF16)
    own2A = nc.dram_tensor("own2A", [NT_A, 128, HC], BF16)
    own2B = nc.dram_tensor("own2B", [NT_B, 128, HC], BF16)
    tab1A = nc.dram_tensor("tab1A", [NCORES, NT_A, 128, HC], BF16,
                           addr_space="Shared")
    tab1B = nc.dram_tensor("tab1B", [NCORES, NT_B, 128, HC], BF16,
                           addr_space="Shared")
    tab2A = nc.dram_tensor("tab2A", [NCORES, NT_A, 128, HC], BF16,
                           addr_space="Shared")
    tab2B = nc.dram_tensor("tab2B", [NCORES, NT_B, 128, HC], BF16,
                           addr_space="Shared")

    with tile.TileContext(nc) as tc:
        with (
            tc.tile_pool(name="const", bufs=1) as cpool,
            tc.tile_pool(name="tab", bufs=3) as tpool,
            tc.tile_pool(name="gath", bufs=3) as gpool,
            tc.tile_pool(name="work", bufs=3) as wpool,
            tc.tile_pool(name="fin", bufs=2) as fpool,
            tc.tile_pool(name="ups", bufs=2, space="PSUM") as psU,
            tc.tile_pool(name="psT", bufs=2, space="PSUM") as psT,
            tc.tile_pool(name="ps2", bufs=2, space="PSUM") as ps2p,
            tc.tile_pool(name="psF", bufs=1, space="PSUM") as psF,
        ):
            def load_const(t, shape, dt):
                tl = cpool.tile(shape, dt, tag=t.name)
                nc.sync.dma_start(out=tl[:], in_=t[:])
                return tl

            w1_sb = load_const(w1cat, [128, 2 * HC], BF16)
            w2_sb = load_const(w2cat, [128, 2, 2 * HC], BF16)
            w3_sb = load_const(w3, [128, 2, 128], BF16)
            w4_sb = load_const(w4, [128, OUT_F], BF16)
            iav1_sb = load_const(iav1, [128, HC], F32)
            iav2_sb = load_const(iav2, [128, HC], F32)
            b1f_sb = load_const(b1f, [128, HC], F32)
            b2f_sb = load_const(b2f, [128, HC], F32)
            b3c_sb = load_const(b3c, [128, 1], F32)
            b4f_sb = load_const(b4f, [128, OUT_F], F32)
            iden_sb = load_const(idenBF, [128, 128], BF16)
            xoT_sb = load_const(xoT, [128, SPC], BF16)
            nbias = cpool.tile([128, 1], F32, tag="nbias")
            nc.vector.memset(nbias[:], -SCB)
            epsc = cpool.tile([128, 1], F32, tag="epsc")
            nc.vector.memset(epsc[:], 1e-16)

            # ---------- L1 tables: per tile [xl1|xr1], own chunks for AllGather
            for t in range(NTILES):
                ps = ps2p.tile([128, 2 * HC], F32, tag="ps2")
                nc.tensor.matmul(ps[:], xoT_sb[:, t * 128:(t + 1) * 128],
                                 w1_sb[:], start=True, stop=True)
                lt = tpool.tile([128, 2, HC], BF16, tag="lt")
                nc.vector.tensor_copy(lt[:, 0, :], ps[:, 0:HC])
                nc.scalar.activation(lt[:, 1, :], ps[:, HC:2 * HC], AF.Copy)
                nc.sync.dma_start(out=loc1[t], in_=lt[:])
                if t < NT_A:
                    nc.scalar.dma_start(out=own1A[t], in_=lt[:, 0, :])
                else:
                    nc.scalar.dma_start(out=own1B[t - NT_A], in_=lt[:, 0, :])

            nc.gpsimd.collective_compute(
                "AllGather", mybir.AluOpType.bypass,
                replica_groups=[list(range(NCORES))],
                ins=[own1A.ap().opt()], outs=[tab1A.ap().opt()])
            nc.gpsimd.collective_compute(
                "AllGather", mybir.AluOpType.bypass,
                replica_groups=[list(range(NCORES))],
                ins=[own1B.ap().opt()], outs=[tab1B.ap().opt()])

            # ---------- conv layer
            qctr = [0]

            def conv(tabA, tabB, loc_tab, lrr, iav_sb, bf_sb, fin):
                tabAf = tabA.rearrange("k t p c -> (k t p) c")
                tabBf = tabB.rearrange("k t p c -> (k t p) c")
                for (t0, nb) in batches:
                    r0 = int(row0[t0])
                    rows = int(row0[t0 + nb] - r0)
                    a0, aN = int(cumTA[t0]), int(cumTA[t0 + nb] - cumTA[t0])
                    b0, bN = int(cumTB[t0]), int(cumTB[t0 + nb] - cumTB[t0])
                    ixa = gpool.tile([128, max(aN, 1) * 8], I16, tag="ixa")
                    if aN:
                        nc.sync.dma_start(
                            out=ixa[:], in_=idxA_d[:, a0 * 8:(a0 + aN) * 8])
                    ixb = gpool.tile([128, max(bN, 1) * 8], I16, tag="ixb")
                    if bN:
                        nc.sync.dma_start(
                            out=ixb[:], in_=idxB_d[:, b0 * 8:(b0 + bN) * 8])
                    vd = gpool.tile([128, rows], FP16, tag="vd")
                    nc.sync.dma_start(out=vd[:], in_=valid_d[:, r0:r0 + rows])
                    sxr = gpool.tile([128, nb, HC], BF16, tag="sxr")
                    nc.sync.dma_start(
                        out=sxr[:],
                        in_=loc_tab[t0:t0 + nb, :, 1, :].rearrange(
                            "t p c -> p t c"))
                    g = gpool.tile([128, rows, HC], BF16, tag="g")
                    for n in range(nb):
                        t = t0 + n
                        ta, tb = int(TA[t]), int(TBp[t])
                        ro = int(row0[t]) - r0
                        # dedicated queues: A->0/1, B->2/3 so B-gathers
                        # blocked on the tab*B collective can't head-of-line
                        # block A-gathers
                        if ta:
                            nc.gpsimd.dma_gather(
                                out_ap=g[:, ro:ro + ta, :], in_ap=tabAf[:],
                                idxs_ap=ixa[:, (int(cumTA[t]) - a0) * 8:
                                            (int(cumTA[t + 1]) - a0) * 8],
                                num_idxs=ta * 128, num_idxs_reg=ta * 128,
                                elem_size=HC, single_packet=False,
                                queue_num=qctr[0] % 2)
                        if tb:
                            nc.gpsimd.dma_gather(
                                out_ap=g[:, ro + ta:ro + ta + tb, :],
                                in_ap=tabBf[:],
                                idxs_ap=ixb[:, (int(cumTB[t]) - b0) * 8:
                                            (int(cumTB[t + 1]) - b0) * 8],
                                num_idxs=tb * 128, num_idxs_reg=tb * 128,
                                elem_size=HC, single_packet=False,
                                queue_num=2 + qctr[0] % 2)
                        qctr[0] += 1
                        # self xl row (last row of tile block)
                        nc.scalar.dma_start(
                            out=g[:, ro + int(ROWS[t]) - 1, :],
                            in_=loc_tab[t, :, 0, :])
                    # ub = g + xr (per tile broadcast), fp16
                    ub = wpool.tile([128, rows, HC], FP16, tag="ub")
                    for n in range(nb):
                        t = t0 + n
                        ro = int(row0[t]) - r0
                        rt = int(ROWS[t])
                        nc.vector.tensor_tensor(
                            out=ub[:, ro:ro + rt, :], in0=g[:, ro:ro + rt, :],
                            in1=sxr[:, n, :].rearrange(
                                "p c -> p () c").broadcast_to([128, rt, HC]),
                            op=OP.add)
                    # leaky-relu on ACT via Prelu (sign-folded ranges)
                    for (c0, c1, kind) in lrr:
                        if kind == "max":
                            nc.scalar.activation(
                                ub[:, :, c0:c1], ub[:, :, c0:c1], AF.Prelu,
                                alpha=SLOPE)
                        else:
                            nc.scalar.activation(
                                ub[:, :, c0:c1], ub[:, :, c0:c1], AF.Prelu,
                                scale=SLOPE, alpha=1.0 / SLOPE)
                    # fold c 128->1 per head (fp16 tree)
                    ubv = ub[:].rearrange("p r (h c) -> p r h c", h=2)
                    for w in (64, 32, 16, 8, 4, 2):
                        nc.vector.tensor_tensor(
                            out=ubv[:, :, :, 0:w], in0=ubv[:, :, :, 0:w],
                            in1=ubv[:, :, :, w:2 * w], op=OP.add)
                    sc = wpool.tile([128, rows, 2], F32, tag="sc")
                    nc.vector.tensor_tensor(
                        out=sc[:].rearrange("p r h -> p r h ()"),
                        in0=ubv[:, :, :, 0:1], in1=ubv[:, :, :, 1:2],
                        op=OP.add)
                    # mask invalid slots then exp
                    nc.vector.scalar_tensor_tensor(
                        out=sc[:], in0=sc[:], scalar=SCB,
                        in1=vd[:].rearrange("p r -> p r ()").broadcast_to(
                            [128, rows, 2]),
                        op0=OP.add, op1=OP.mult)
                    afb = wpool.tile([128, rows, 2], BF16, tag="afb")
                    nc.scalar.activation(afb[:], sc[:], AF.Exp, bias=nbias[:])
                    # denominators per tile
                    den = wpool.tile([128, nb, 2], F32, tag="den")
                    for n in range(nb):
                        t = t0 + n
                        ro = int(row0[t]) - r0
                        nc.vector.tensor_reduce(
                            out=den[:, n, :].rearrange("p h -> p h ()"),
                            in_=afb[:, ro:ro + int(ROWS[t]), :].rearrange(
                                "p t h -> p h t"),
                            axis=mybir.AxisListType.X, op=OP.add)
                    nc.scalar.activation(den[:], den[:], AF.Identity,
                                         bias=epsc[:])
                    rden = wpool.tile([128, nb, 2], F32, tag="rden")
                    nc.vector.reciprocal(rden[:], den[:])
                    # ya = alpha * xl
                    yav = ub[:].bitcast(BF16)
                    nc.vector.tensor_tensor(
                        out=yav.rearrange("p r (h c) -> p r h c", h=2),
                        in0=g[:].rearrange("p r (h c) -> p r h c", h=2),
                        in1=afb[:].rearrange("p r h -> p r h ()").broadcast_to(
                            [128, rows, 2, 128]),
                        op=OP.mult)
                    # aggregate per tile: identity matmuls, 512-wide pairs
                    yf = yav.rearrange("p r c -> p (r c)")
                    for n in range(nb):
                        t = t0 + n
                        ro = int(row0[t]) - r0
                        npair = int(ROWS[t]) // 2
                        ups = psU.tile([128, 512], F32, tag="ups")
                        for j in range(npair):
                            o = (ro + 2 * j) * HC
                            nc.tensor.matmul(
                                ups[:], iden_sb[:], yf[:, o:o + 512],
                                start=(j == 0), stop=(j == npair - 1))
                        fin(t, ups, rden, n, iav_sb, bf_sb)

            # ---------- finalize helpers
            def head_merge(ups, rden, n, iav_sb, bf_sb):
                upB = fpool.tile([128, HC], F32, tag="upB")
                nc.scalar.activation(upB[:], ups[:, HC:2 * HC], AF.Copy)
                u = fpool.tile([128, HC], F32, tag="u")
                nc.vector.tensor_tensor(
                    out=u[:], in0=ups[:, 0:HC], in1=upB[:], op=OP.add)
                nc.vector.tensor_tensor(
                    out=u[:].rearrange("p (h c) -> p h c", h=2),
                    in0=u[:].rearrange("p (h c) -> p h c", h=2),
                    in1=rden[:, n, :].rearrange("p h -> p h ()").broadcast_to(
                        [128, 2, 128]),
                    op=OP.mult)
                nc.vector.tensor_tensor(out=u[:], in0=u[:], in1=iav_sb[:],
                                        op=OP.mult)
                nc.vector.tensor_tensor(out=u[:], in0=u[:], in1=bf_sb[:],
                                        op=OP.add)
                hbf = fpool.tile([128, HC], BF16, tag="hbf")
                nc.scalar.activation(hbf[:], u[:], AF.Relu)
                ct = fpool.tile([128, 2, 128], BF16, tag="ct")
                for h in range(2):
                    pt = psT.tile([128, 128], BF16, tag="pt")
                    nc.tensor.transpose(pt[:], hbf[:, h * 128:(h + 1) * 128],
                                        iden_sb[:])
                    if h == 0:
                        nc.scalar.activation(ct[:, h, :], pt[:], AF.Copy)
                    else:
                        nc.vector.tensor_copy(ct[:, h, :], pt[:])
                return ct

            def fin1(t, ups, rden, n, iav_sb, bf_sb):
                ct = head_merge(ups, rden, n, iav_sb, bf_sb)
                ps2 = ps2p.tile([128, 2 * HC], F32, tag="ps2")
                nc.tensor.matmul(ps2[:], ct[:, 0, :], w2_sb[:, 0, :],
                                 start=True, stop=False)
                nc.tensor.matmul(ps2[:], ct[:, 1, :], w2_sb[:, 1, :],
                                 start=False, stop=True)
                l2t = fpool.tile([128, 2, HC], BF16, tag="l2t")
                nc.vector.tensor_copy(l2t[:, 0, :], ps2[:, 0:HC])
                nc.scalar.activation(l2t[:, 1, :], ps2[:, HC:2 * HC], AF.Copy)
                nc.sync.dma_start(out=loc2[t], in_=l2t[:])
                if t < NT_A:
                    nc.scalar.dma_start(out=own2A[t], in_=l2t[:, 0, :])
                else:
                    nc.scalar.dma_start(out=own2B[t - NT_A], in_=l2t[:, 0, :])

            def fin2(t, ups, rden, n, iav_sb, bf_sb):
                ct = head_merge(ups, rden, n, iav_sb, bf_sb)
                zt_ps = psF.tile([128, 128], F32, tag="ztps")
                nc.tensor.matmul(zt_ps[:], w3_sb[:, 0, :], ct[:, 0, :],
                                 start=True, stop=False)
                nc.tensor.matmul(zt_ps[:], w3_sb[:, 1, :], ct[:, 1, :],
                                 start=False, stop=True)
                zt = fpool.tile([128, 128], BF16, tag="zt")
                nc.scalar.activation(zt[:], zt_ps[:], AF.Identity,
                                     bias=b3c_sb[:], scale=1.0)
                o_ps = psF.tile([128, OUT_F], F32, tag="ops")
                nc.tensor.matmul(o_ps[:], zt[:], w4_sb[:], start=True,
                                 stop=True)
                o_pre = fpool.tile([128, OUT_F], F32, tag="opre")
                nc.vector.scalar_tensor_tensor(
                    out=o_pre[:], in0=o_ps[:], scalar=1.0, in1=b4f_sb[:],
                    op0=OP.mult, op1=OP.add)
                o_sb = fpool.tile([128, OUT_F], F32, tag="osb")
                nc.scalar.activation(o_sb[:], o_pre[:], AF.Sigmoid)
                nc.sync.dma_start(out=out_ext[t * 128:(t + 1) * 128, :],
                                  in_=o_sb[:])

            # ================= phase schedule =================
            import os as _os
            _upto = int(_os.environ.get("KPHASES", "9"))
            if _upto >= 2:
                conv(tab1A, tab1B, loc1, LR1, iav1_sb, b1f_sb, fin1)
            if _upto >= 3:
                nc.gpsimd.collective_compute(
                    "AllGather", mybir.AluOpType.bypass,
                    replica_groups=[list(range(NCORES))],
                    ins=[own2A.ap().opt()], outs=[tab2A.ap().opt()])
                nc.gpsimd.collective_compute(
                    "AllGather", mybir.AluOpType.bypass,
                    replica_groups=[list(range(NCORES))],
                    ins=[own2B.ap().opt()], outs=[tab2B.ap().opt()])
            if _upto >= 4:
                conv(tab2A, tab2B, loc2, LR2, iav2_sb, b2f_sb, fin2)
            else:
                zt = fpool.tile([128, OUT_F], F32, tag="osb")
                nc.vector.memset(zt[:], 0.0)
                for t in range(NTILES):
                    nc.sync.dma_start(out=out_ext[t * 128:(t + 1) * 128, :],
                                      in_=zt[:])

    nc.compile()
    return nc


# ---------------------------------------------------------------- entry point
def kernel(**inputs):
    from concourse import bass_utils

    src = np.asarray(inputs["edge_index"][0], np.int64)
    dst = np.asarray(inputs["edge_index"][1], np.int64)
    x = np.asarray(inputs["x"], np.float32)

    pack = _pack_graph(src, dst)
    nos = pack["node_of_slot"]
    valid_slot = nos >= 0
    x_slot = np.zeros((S, IN_F), np.float32)
    x_slot[valid_slot] = x[nos[valid_slot]]

    def bf(a):
        return np.ascontiguousarray(np.asarray(a, np.float32)).astype(BF)

    # per-head column permutation (+att cols first) + pre-scale by att
    def prep_layer(att):
        att = np.asarray(att, np.float32).reshape(2, 128)
        perm = np.zeros(HC, np.int64)
        ranges = []
        for h in range(2):
            a = att[h]
            pos = np.where(a > 0)[0]
            neg = np.where(a <= 0)[0]
            perm[h * 128:(h + 1) * 128] = h * 128 + np.concatenate([pos, neg])
            p = len(pos)
            if p:
                ranges.append((h * 128, h * 128 + p, "max"))
            if p < 128:
                ranges.append((h * 128 + p, (h + 1) * 128, "min"))
        att_p = att.reshape(HC)[perm]
        att_p = np.where(np.abs(att_p) < 1e-30, 1e-30, att_p)
        return perm, att_p, ranges

    perm1, att1p, LR1 = prep_layer(inputs["att1"])
    perm2, att2p, LR2 = prep_layer(inputs["att2"])
    _PLAN["LR1"] = LR1
    _PLAN["LR2"] = LR2

    Wl1p = np.asarray(inputs["Wl1"], np.float32)[:, perm1] * att1p[None, :]
    Wr1p = np.asarray(inputs["Wr1"], np.float32)[:, perm1] * att1p[None, :]
    Wl2p = (np.asarray(inputs["Wl2"], np.float32)[perm1][:, perm2]
            * att2p[None, :])
    Wr2p = (np.asarray(inputs["Wr2"], np.float32)[perm1][:, perm2]
            * att2p[None, :])
    W3p = np.asarray(inputs["W3"], np.float32)[perm2]
    b1p = np.asarray(inputs["b1"], np.float32)[perm1]
    b2p = np.asarray(inputs["b2"], np.float32)[perm2]

    w2c = np.concatenate([Wl2p, Wr2p], 1)           # [256, 512]
    common = {
        "w1cat": bf(np.concatenate([Wl1p, Wr1p], 1)),
        "w2cat": bf(w2c.reshape(2, 128, 2 * HC).transpose(1, 0, 2)),
        "w3": bf(W3p.reshape(2, 128, 128).transpose(1, 0, 2)),
        "w4": bf(inputs["W4"]),
        "iav1": np.tile((1.0 / att1p)[None, :], (128, 1)).astype(np.float32),
        "iav2": np.tile((1.0 / att2p)[None, :], (128, 1)).astype(np.float32),
        "b1f": np.tile(b1p[None, :], (128, 1)),
        "b2f": np.tile(b2p[None, :], (128, 1)),
        "b3c": np.asarray(inputs["b3"], np.float32).reshape(128, 1),
        "b4f": np.tile(np.asarray(inputs["b4"], np.float32)[None, :],
                       (128, 1)),
        "idenBF": np.eye(128, dtype=np.float32).astype(BF),
    }

    in_maps = []
    for k in range(NCORES):
        m = dict(common)
        m["xoT"] = np.ascontiguousarray(
            x_slot[k * SPC:(k + 1) * SPC].T).astype(BF)
        m["idxA_d"] = pack["idxA_d"][k]
        m["idxB_d"] = pack["idxB_d"][k]
        m["valid_d"] = pack["valid_d"][k]
        in_maps.append(m)

    if "nc" not in _NC_CACHE:
        _NC_CACHE["nc"] = _build_nc()
    nc = _NC_CACHE["nc"]

    res = bass_utils.run_bass_kernel_spmd(nc, in_maps,
                                          core_ids=list(range(NCORES)),
                                          **_RUN_OPTS)
    _LAST_RESULTS["res"] = res
    out_slots = np.concatenate([res.results[k]["out"] for k in range(NCORES)],
                               0)
    return out_slots[pack["slot_of_node"]].astype(np.float32)


# revision 15
# speedup vs baseline: 1.0914x; 1.0625x over previous
"""GATv2 (2-layer, 2-head) Trainium2 kernel, 8-core SPMD — lane-aligned v2.

Strategy: dst-node partition across 8 cores. Host assigns nodes to
(core, tile, lane) with a half-preserving two-pass packing: pass 1 sorts by
in-degree (fixes which gather half each node's slot is in), pass 2 re-sorts
within each half by (degA, degB) so tiles have uniform per-lane edge counts.
Edge slots are LANE-ALIGNED (slot partition == dst lane), so aggregation is
a plain identity-matmul accumulation over subtiles (512-wide PSUM pairs) and
xr[dst] is a per-tile broadcast — no per-edge xr gather, no mask build.
Scores: Prelu on ACT (column-sign handled by Prelu(u,.2)/Prelu(.2u,5)),
fp16 binary-tree folds on DVE, exp with -1024 bias masks invalid slots.
Full xl tables are built shard-wise and AllGathered in two chunks (A/B)
whose boundary doubles as the int16 gather-index split; layer-2 local
tables (loc2/own2) are fused into layer-1 finalize. Dense tail fused into
layer-2 finalize.
"""
import sys

sys.path.insert(0, "/opt/trn_rl_repo")

import numpy as np
import ml_dtypes

BF = ml_dtypes.bfloat16
F16 = np.float16

# ---- static layout constants ----
N = 50000
NCORES = 8
LANES = 128
NTILES = 49
SPC = NTILES * LANES          # 6272 slots per core
S = NCORES * SPC
NT_A = 25                     # tiles 0..24 -> gather half A
NT_B = 24
CHA = NT_A * LANES            # 3200
CHB = NT_B * LANES            # 3072
RA = NCORES * CHA             # 25600 rows in tabA (int16-safe)
RB = NCORES * CHB             # 24576
IN_F = 128
HC = 256
OUT_F = 40
SLOPE = 0.2
ROWB = 48                     # batch row budget (SBUF-bound)
SCB = 1024.0                  # score bias for invalid-slot masking

_NC_CACHE = {}
_RUN_OPTS = {}
_LAST_RESULTS = {}
_PLAN = {}


# ---------------------------------------------------------------- host prep
def _wrap_idx(idx):
    """[n] -> [128, n//16] wrapped (j at partition j%16, col j//16) + replicated."""
    n = idx.shape[0]
    a = idx.reshape(n // 16, 16).T.astype(np.int16)
    return np.tile(a, (8, 1))


def _pack_graph(src, dst):
    # peel one self-loop per node (all nodes have one: loops appended)
    is_self = src == dst
    self_eids = np.full(N, -1, np.int64)
    sids = np.where(is_self)[0]
    self_eids[src[sids]] = sids
    rand_mask = np.ones(len(src), bool)
    rand_mask[self_eids[self_eids >= 0]] = False
    rsrc, rdst = src[rand_mask], dst[rand_mask]

    deg = np.bincount(rdst, minlength=N)

    # pass 1: global deg sort, deal round-robin to cores, chunk -> halves
    order = np.argsort(-deg, kind="stable")
    core_of = np.empty(N, np.int32)
    core_of[order] = np.arange(N) % NCORES
    local1 = np.empty(N, np.int64)
    for k in range(NCORES):
        vs = order[core_of[order] == k]
        local1[vs] = np.arange(len(vs))
    halfA_node = local1 < CHA

    eA = halfA_node[rsrc]
    degA = np.bincount(rdst[eA], minlength=N)
    degB = deg - degA

    # pass 2: within each half sort by (degA, degB), chunk into that half's
    # tiles; lane = position within chunk
    tile_of_node = np.full(N, -1, np.int32)
    lane_of_node = np.full(N, -1, np.int32)
    for k in range(NCORES):
        vs = np.where(core_of == k)[0]
        A = vs[halfA_node[vs]]
        Bn = vs[~halfA_node[vs]]
        A = A[np.argsort(-(degA[A] * 1000 + degB[A]), kind="stable")]
        Bn = Bn[np.argsort(-(degA[Bn] * 1000 + degB[Bn]), kind="stable")]
        tile_of_node[A] = np.arange(len(A)) // LANES
        lane_of_node[A] = np.arange(len(A)) % LANES
        tile_of_node[Bn] = NT_A + np.arange(len(Bn)) // LANES
        lane_of_node[Bn] = np.arange(len(Bn)) % LANES

    slot_of_node = (core_of.astype(np.int64) * SPC
                    + tile_of_node.astype(np.int64) * LANES + lane_of_node)
    node_of_slot = np.full(S, -1, np.int64)
    node_of_slot[slot_of_node] = np.arange(N)

    # per-slot A/B counts -> common per-tile profile
    dstslot = slot_of_node[rdst]
    srcslot = slot_of_node[rsrc]
    loc_s = srcslot % SPC
    isB = loc_s >= CHA
    cntA = np.bincount(dstslot[~isB], minlength=S).reshape(NCORES, NTILES, 128)
    cntB = np.bincount(dstslot[isB], minlength=S).reshape(NCORES, NTILES, 128)
    TA = cntA.max(axis=(0, 2)).astype(np.int64)     # [NTILES] common
    TB = cntB.max(axis=(0, 2)).astype(np.int64)
    TBp = TB + (TA + TB + 1) % 2                    # ROWS even
    ROWS = TA + TBp + 1
    cumTA = np.concatenate([[0], np.cumsum(TA)])
    cumTB = np.concatenate([[0], np.cumsum(TBp)])
    row0 = np.concatenate([[0], np.cumsum(ROWS)])
    SUMA, SUMB, SUMR = int(cumTA[-1]), int(cumTB[-1]), int(row0[-1])

    # processing order: interleave half-chunks A1 B1 A2 B2 so the own-chunk
    # collectives fire at 25/50/75/100%% of the conv and the last one is small
    CH = [list(range(0, 13)), list(range(NT_A, NT_A + 12)),
          list(range(13, NT_A)), list(range(NT_A + 12, NTILES))]
    seq = [t for ch in CH for t in ch]
    # seq-order layouts
    TAs = TA[seq]
    TBs = TBp[seq]
    ROWSs = ROWS[seq]
    cumTA = np.concatenate([[0], np.cumsum(TAs)])
    cumTB = np.concatenate([[0], np.cumsum(TBs)])
    row0 = np.concatenate([[0], np.cumsum(ROWSs)])
    pos_of_tile = np.empty(NTILES, np.int64)
    pos_of_tile[seq] = np.arange(NTILES)

    # batches: consecutive seq positions within row budget, not crossing
    # chunk boundaries
    bounds = set(np.cumsum([len(c) for c in CH]).tolist())
    batches = []
    cur0, acc = 0, 0
    for p in range(NTILES):
        if (acc + ROWSs[p] > ROWB or p in bounds) and acc > 0:
            batches.append((cur0, p - cur0))
            cur0, acc = p, 0
        acc += ROWSs[p]
    batches.append((cur0, NTILES - cur0))

    # per-edge gather index: chunk-major tables
    # A: [A1: 13 tiles][A2: 12 tiles], B: [B1: 12][B2: 12]; within a chunk
    # rows are (core, tile-in-chunk, lane)
    ks = srcslot // SPC
    st = loc_s // LANES
    sl = loc_s % LANES
    gidx = np.where(
        st < 13, ks * (13 * 128) + st * 128 + sl,
        np.where(st < 25,
                 13 * 8 * 128 + ks * (12 * 128) + (st - 13) * 128 + sl,
                 np.where(st < 37,
                          ks * (12 * 128) + (st - 25) * 128 + sl,
                          12 * 8 * 128 + ks * (12 * 128)
                          + (st - 37) * 128 + sl))).astype(np.int64)
    kc = (dstslot // SPC).astype(np.int64)
    kt = ((dstslot % SPC) // LANES).astype(np.int64)
    kl = (dstslot % LANES).astype(np.int64)

    # rank within (dstslot, half)
    key = dstslot * 2 + isB
    es = np.argsort(key, kind="stable")
    gkey = key[es]
    start = np.ones(len(es), bool)
    start[1:] = gkey[1:] != gkey[:-1]
    gs = np.where(start, np.arange(len(es)), 0)
    rank = np.arange(len(es)) - np.maximum.accumulate(gs)

    e_kc = kc[es]
    e_kt = kt[es]
    e_kl = kl[es]
    e_isB = isB[es]
    e_gidx = gidx[es]

    idxA = np.zeros((NCORES, SUMA, 128), np.int16)
    idxB = np.zeros((NCORES, SUMB, 128), np.int16)
    valid = np.zeros((NCORES, SUMR, 128), F16)

    mA = ~e_isB
    pA = pos_of_tile[e_kt[mA]]
    idxA[e_kc[mA], cumTA[pA] + rank[mA], e_kl[mA]] = e_gidx[mA]
    valid[e_kc[mA], row0[pA] + rank[mA], e_kl[mA]] = 1.0
    mB = e_isB
    pB = pos_of_tile[e_kt[mB]]
    idxB[e_kc[mB], cumTB[pB] + rank[mB], e_kl[mB]] = e_gidx[mB]
    valid[e_kc[mB], row0[pB] + TAs[pB] + rank[mB], e_kl[mB]] = 1.0
    # self rows (last row of each tile block), all real nodes
    pn = pos_of_tile[tile_of_node]
    valid[core_of, row0[pn] + ROWSs[pn] - 1, lane_of_node] = 1.0

    # wrapped idx tensors [NCORES, 128, SUM*8]
    idxA_d = np.empty((NCORES, 128, SUMA * 8), np.int16)
    idxB_d = np.empty((NCORES, 128, SUMB * 8), np.int16)
    for k in range(NCORES):
        for p in range(NTILES):
            if TAs[p]:
                idxA_d[k][:, cumTA[p] * 8:cumTA[p + 1] * 8] = _wrap_idx(
                    idxA[k, cumTA[p]:cumTA[p + 1]].reshape(-1))
            if TBs[p]:
                idxB_d[k][:, cumTB[p] * 8:cumTB[p + 1] * 8] = _wrap_idx(
                    idxB[k, cumTB[p]:cumTB[p + 1]].reshape(-1))
    valid_d = np.ascontiguousarray(valid.transpose(0, 2, 1))  # [NC,128,SUMR]

    _PLAN.update(TA=TAs, TBp=TBs, ROWS=ROWSs, cumTA=cumTA, cumTB=cumTB,
                 row0=row0, SUMA=SUMA, SUMB=SUMB, SUMR=SUMR, batches=batches,
                 seq=seq, CH=CH)

    return dict(slot_of_node=slot_of_node, node_of_slot=node_of_slot,
                idxA_d=idxA_d, idxB_d=idxB_d, valid_d=valid_d)


# ---------------------------------------------------------------- device kernel
def _build_nc():
    import concourse.bass as bass
    import concourse.bacc as bacc
    import concourse.tile as tile
    import concourse.mybir as mybir

    F32 = mybir.dt.float32
    BF16 = mybir.dt.bfloat16
    FP16 = mybir.dt.float16
    I16 = mybir.dt.int16
    AF = mybir.ActivationFunctionType
    OP = mybir.AluOpType

    LR1, LR2 = _PLAN["LR1"], _PLAN["LR2"]
    TA, TBp, ROWS = _PLAN["TA"], _PLAN["TBp"], _PLAN["ROWS"]
    cumTA, cumTB, row0 = _PLAN["cumTA"], _PLAN["cumTB"], _PLAN["row0"]
    SUMA, SUMB, SUMR = _PLAN["SUMA"], _PLAN["SUMB"], _PLAN["SUMR"]
    batches = _PLAN["batches"]
    seq, CH = _PLAN["seq"], _PLAN["CH"]

    nc = bacc.Bacc(None, target_bir_lowering=False, num_swdge_queues=4)

    # ---- inputs
    xoT = nc.dram_tensor("xoT", [128, SPC], BF16, kind="ExternalInput")
    w1cat = nc.dram_tensor("w1cat", [128, 2 * HC], BF16, kind="ExternalInput")
    w2cat = nc.dram_tensor("w2cat", [128, 2, 2 * HC], BF16,
                           kind="ExternalInput")
    w3 = nc.dram_tensor("w3", [128, 2, 128], BF16, kind="ExternalInput")
    w4 = nc.dram_tensor("w4", [128, OUT_F], BF16, kind="ExternalInput")
    iavT1 = nc.dram_tensor("iavT1", [128, 2], F32, kind="ExternalInput")
    iavT2 = nc.dram_tensor("iavT2", [128, 2], F32, kind="ExternalInput")
    bT1 = nc.dram_tensor("bT1", [128, 2], F32, kind="ExternalInput")
    bT2 = nc.dram_tensor("bT2", [128, 2], F32, kind="ExternalInput")
    b3c = nc.dram_tensor("b3c", [128, 1], F32, kind="ExternalInput")
    b4f = nc.dram_tensor("b4f", [128, OUT_F], F32, kind="ExternalInput")
    idenBF = nc.dram_tensor("idenBF", [128, 128], BF16, kind="ExternalInput")
    idxA_d = nc.dram_tensor("idxA_d", [128, SUMA * 8], I16,
                            kind="ExternalInput")
    idxB_d = nc.dram_tensor("idxB_d", [128, SUMB * 8], I16,
                            kind="ExternalInput")
    valid_d = nc.dram_tensor("valid_d", [128, SUMR], FP16,
                             kind="ExternalInput")
    out_ext = nc.dram_tensor("out", [SPC, OUT_F], F32, kind="ExternalOutput")

    # ---- DRAM intermediates
    loc1 = nc.dram_tensor("loc1", [NTILES, 128, 2, HC], BF16)
    loc2 = nc.dram_tensor("loc2", [NTILES, 128, 2, HC], BF16)
    own1A = nc.dram_tensor("own1A", [NT_A, 128, HC], BF16)
    own1B = nc.dram_tensor("own1B", [NT_B, 128, HC], BF16)
    own2A = nc.dram_tensor("own2A", [NT_A, 128, HC], BF16)
    own2B = nc.dram_tensor("own2B", [NT_B, 128, HC], BF16)
    tab1A = nc.dram_tensor("tab1A", [RA, HC], BF16, addr_space="Shared")
    tab1B = nc.dram_tensor("tab1B", [RB, HC], BF16, addr_space="Shared")
    tab2A = nc.dram_tensor("tab2A", [RA, HC], BF16, addr_space="Shared")
    tab2B = nc.dram_tensor("tab2B", [RB, HC], BF16, addr_space="Shared")

    with tile.TileContext(nc) as tc:
        with (
            tc.tile_pool(name="const", bufs=1) as cpool,
            tc.tile_pool(name="tab", bufs=3) as tpool,
            tc.tile_pool(name="gath", bufs=3) as gpool,
            tc.tile_pool(name="work", bufs=3) as wpool,
            tc.tile_pool(name="fin", bufs=2) as fpool,
            tc.tile_pool(name="ups", bufs=2, space="PSUM") as psU,
            tc.tile_pool(name="psT", bufs=2, space="PSUM") as psT,
            tc.tile_pool(name="ps2", bufs=2, space="PSUM") as ps2p,
            tc.tile_pool(name="psF", bufs=1, space="PSUM") as psF,
        ):
            def load_const(t, shape, dt):
                tl = cpool.tile(shape, dt, tag=t.name)
                nc.sync.dma_start(out=tl[:], in_=t[:])
                return tl

            w1_sb = load_const(w1cat, [128, 2 * HC], BF16)
            w2_sb = load_const(w2cat, [128, 2, 2 * HC], BF16)
            w3_sb = load_const(w3, [128, 2, 128], BF16)
            w4_sb = load_const(w4, [128, OUT_F], BF16)
            iavT1_sb = load_const(iavT1, [128, 2], F32)
            iavT2_sb = load_const(iavT2, [128, 2], F32)
            bT1_sb = load_const(bT1, [128, 2], F32)
            bT2_sb = load_const(bT2, [128, 2], F32)
            b3c_sb = load_const(b3c, [128, 1], F32)
            b4f_sb = load_const(b4f, [128, OUT_F], F32)
            iden_sb = load_const(idenBF, [128, 128], BF16)
            xoT_sb = load_const(xoT, [128, SPC], BF16)
            nbias = cpool.tile([128, 1], F32, tag="nbias")
            nc.vector.memset(nbias[:], -SCB)
            epsc = cpool.tile([128, 1], F32, tag="epsc")
            nc.vector.memset(epsc[:], 1e-16)

            # ---------- L1 tables (seq order) + chunked AllGathers
            def chunk_cc(ownA, ownB, tabA, tabB, ci):
                # chunk-major tables: chunk ci occupies a contiguous row
                # range; AllGather lays cores core-major within it
                ch = CH[ci]
                lo, hi = ch[0], ch[-1] + 1
                if lo < NT_A:
                    own, tab, l, h = ownA, tabA, lo, hi
                    r0 = 0 if l == 0 else 13 * NCORES * 128
                else:
                    own, tab, l, h = ownB, tabB, lo - NT_A, hi - NT_A
                    r0 = 0 if l == 0 else 12 * NCORES * 128
                nrows = (h - l) * NCORES * 128
                nc.gpsimd.collective_compute(
                    "AllGather", mybir.AluOpType.bypass,
                    replica_groups=[list(range(NCORES))],
                    ins=[own[l:h].opt()],
                    outs=[tab[r0:r0 + nrows, :].opt()])

            for ci, ch in enumerate(CH):
                for t in ch:
                    ps = ps2p.tile([128, 2 * HC], F32, tag="ps2")
                    nc.tensor.matmul(ps[:], xoT_sb[:, t * 128:(t + 1) * 128],
                                     w1_sb[:], start=True, stop=True)
                    lt = tpool.tile([128, 2, HC], BF16, tag="lt")
                    nc.vector.tensor_copy(lt[:, 0, :], ps[:, 0:HC])
                    nc.scalar.activation(lt[:, 1, :], ps[:, HC:2 * HC],
                                         AF.Copy)
                    nc.sync.dma_start(out=loc1[t], in_=lt[:])
                    if t < NT_A:
                        nc.scalar.dma_start(out=own1A[t], in_=lt[:, 0, :])
                    else:
                        nc.scalar.dma_start(out=own1B[t - NT_A],
                                            in_=lt[:, 0, :])
                chunk_cc(own1A, own1B, tab1A, tab1B, ci)

            # ---------- conv layer
            qctr = [0]

            def conv(tabA, tabB, loc_tab, lrr, iav_sb, bf_sb, fin,
                     cc_next=None):
                tabAf = tabA
                tabBf = tabB
                chunk_ends = np.cumsum([len(c) for c in CH]).tolist()
                for (p0, nb) in batches:
                    t0 = seq[p0]
                    r0 = int(row0[p0])
                    rows = int(row0[p0 + nb] - r0)
                    a0, aN = int(cumTA[p0]), int(cumTA[p0 + nb] - cumTA[p0])
                    b0, bN = int(cumTB[p0]), int(cumTB[p0 + nb] - cumTB[p0])
                    ixa = gpool.tile([128, max(aN, 1) * 8], I16, tag="ixa")
                    if aN:
                        nc.sync.dma_start(
                            out=ixa[:], in_=idxA_d[:, a0 * 8:(a0 + aN) * 8])
                    ixb = gpool.tile([128, max(bN, 1) * 8], I16, tag="ixb")
                    if bN:
                        nc.sync.dma_start(
                            out=ixb[:], in_=idxB_d[:, b0 * 8:(b0 + bN) * 8])
                    vd = gpool.tile([128, rows], FP16, tag="vd")
                    nc.sync.dma_start(out=vd[:], in_=valid_d[:, r0:r0 + rows])
                    sxr = gpool.tile([128, nb, HC], BF16, tag="sxr")
                    nc.sync.dma_start(
                        out=sxr[:],
                        in_=loc_tab[t0:t0 + nb, :, 1, :].rearrange(
                            "t p c -> p t c"))
                    g = gpool.tile([128, rows, HC], BF16, tag="g")
                    for n in range(nb):
                        p = p0 + n
                        t = t0 + n
                        ta, tb = int(TA[p]), int(TBp[p])
                        ro = int(row0[p]) - r0
                        # dedicated queues: A->0/1, B->2/3 so B-gathers
                        # blocked on the tab*B collective can't head-of-line
                        # block A-gathers
                        if ta:
                            nc.gpsimd.dma_gather(
                                out_ap=g[:, ro:ro + ta, :], in_ap=tabAf[:],
                                idxs_ap=ixa[:, (int(cumTA[p]) - a0) * 8:
                                            (int(cumTA[p + 1]) - a0) * 8],
                                num_idxs=ta * 128, num_idxs_reg=ta * 128,
                                elem_size=HC, single_packet=False,
                                queue_num=qctr[0] % 2)
                        if tb:
                            nc.gpsimd.dma_gather(
                                out_ap=g[:, ro + ta:ro + ta + tb, :],
                                in_ap=tabBf[:],
                                idxs_ap=ixb[:, (int(cumTB[p]) - b0) * 8:
                                            (int(cumTB[p + 1]) - b0) * 8],
                                num_idxs=tb * 128, num_idxs_reg=tb * 128,
                                elem_size=HC, single_packet=False,
                                queue_num=2 + qctr[0] % 2)
                        qctr[0] += 1
                        # self xl row (last row of tile block)
                        nc.scalar.dma_start(
                            out=g[:, ro + int(ROWS[p]) - 1, :],
                            in_=loc_tab[t, :, 0, :])
                    # ub = g + xr (per tile broadcast), fp16
                    ub = wpool.tile([128, rows, HC], FP16, tag="ub")
                    for n in range(nb):
                        p = p0 + n
                        ro = int(row0[p]) - r0
                        rt = int(ROWS[p])
                        nc.vector.tensor_tensor(
                            out=ub[:, ro:ro + rt, :], in0=g[:, ro:ro + rt, :],
                            in1=sxr[:, n, :].rearrange(
                                "p c -> p () c").broadcast_to([128, rt, HC]),
                            op=OP.add)
                    # leaky-relu on ACT via Prelu (sign-folded ranges)
                    for (c0, c1, kind) in lrr:
                        if kind == "max":
                            nc.scalar.activation(
                                ub[:, :, c0:c1], ub[:, :, c0:c1], AF.Prelu,
                                alpha=SLOPE)
                        else:
                            nc.scalar.activation(
                                ub[:, :, c0:c1], ub[:, :, c0:c1], AF.Prelu,
                                scale=SLOPE, alpha=1.0 / SLOPE)
                    # fold c 128->1 per head (fp16 tree)
                    ubv = ub[:].rearrange("p r (h c) -> p r h c", h=2)
                    for w in (64, 32, 16, 8, 4, 2):
                        nc.vector.tensor_tensor(
                            out=ubv[:, :, :, 0:w], in0=ubv[:, :, :, 0:w],
                            in1=ubv[:, :, :, w:2 * w], op=OP.add)
                    sc = wpool.tile([128, rows, 2], F32, tag="sc")
                    nc.vector.tensor_tensor(
                        out=sc[:].rearrange("p r h -> p r h ()"),
                        in0=ubv[:, :, :, 0:1], in1=ubv[:, :, :, 1:2],
                        op=OP.add)
                    # mask invalid slots then exp
                    nc.vector.scalar_tensor_tensor(
                        out=sc[:], in0=sc[:], scalar=SCB,
                        in1=vd[:].rearrange("p r -> p r ()").broadcast_to(
                            [128, rows, 2]),
                        op0=OP.add, op1=OP.mult)
                    afb = wpool.tile([128, rows, 2], BF16, tag="afb")
                    nc.scalar.activation(afb[:], sc[:], AF.Exp, bias=nbias[:])
                    # denominators per tile
                    den = wpool.tile([128, nb, 2], F32, tag="den")
                    for n in range(nb):
                        p = p0 + n
                        ro = int(row0[p]) - r0
                        nc.vector.tensor_reduce(
                            out=den[:, n, :].rearrange("p h -> p h ()"),
                            in_=afb[:, ro:ro + int(ROWS[p]), :].rearrange(
                                "p t h -> p h t"),
                            axis=mybir.AxisListType.X, op=OP.add)
                    nc.scalar.activation(den[:], den[:], AF.Identity,
                                         bias=epsc[:])
                    rden = wpool.tile([128, nb, 2], F32, tag="rden")
                    nc.vector.reciprocal(rden[:], den[:])
                    # ya = alpha * xl
                    yav = ub[:].bitcast(BF16)
                    nc.vector.tensor_tensor(
                        out=yav.rearrange("p r (h c) -> p r h c", h=2),
                        in0=g[:].rearrange("p r (h c) -> p r h c", h=2),
                        in1=afb[:].rearrange("p r h -> p r h ()").broadcast_to(
                            [128, rows, 2, 128]),
                        op=OP.mult)
                    # aggregate per tile: identity matmuls, 512-wide pairs
                    yf = yav.rearrange("p r c -> p (r c)")
                    for n in range(nb):
                        p = p0 + n
                        t = t0 + n
                        ro = int(row0[p]) - r0
                        npair = int(ROWS[p]) // 2
                        ups = psU.tile([128, 512], F32, tag="ups")
                        for j in range(npair):
                            o = (ro + 2 * j) * HC
                            nc.tensor.matmul(
                                ups[:], iden_sb[:], yf[:, o:o + 512],
                                start=(j == 0), stop=(j == npair - 1))
                        fin(t, ups, rden, n, iav_sb, bf_sb)
                    if cc_next is not None and (p0 + nb) in chunk_ends:
                        cc_next(chunk_ends.index(p0 + nb))

            # ---------- finalize helpers
            def head_merge(ups, rden, n, iavT_sb, bT_sb):
                # u = (psA + psB) * rden_h; iav/bias/relu fused into the
                # post-transpose ACT copies (per-partition col args there)
                upB = fpool.tile([128, HC], F32, tag="upB")
                nc.scalar.activation(upB[:], ups[:, HC:2 * HC], AF.Copy)
                u = fpool.tile([128, HC], F32, tag="u")
                nc.vector.tensor_tensor(
                    out=u[:], in0=ups[:, 0:HC], in1=upB[:], op=OP.add)
                nc.vector.tensor_tensor(
                    out=u[:].rearrange("p (h c) -> p h c", h=2),
                    in0=u[:].rearrange("p (h c) -> p h c", h=2),
                    in1=rden[:, n, :].rearrange("p h -> p h ()").broadcast_to(
                        [128, 2, 128]),
                    op=OP.mult)
                hbf = fpool.tile([128, HC], BF16, tag="hbf")
                nc.scalar.activation(hbf[:], u[:], AF.Copy)
                ct = fpool.tile([128, 2, 128], BF16, tag="ct")
                for h in range(2):
                    pt = psT.tile([128, 128], BF16, tag="pt")
                    nc.tensor.transpose(pt[:], hbf[:, h * 128:(h + 1) * 128],
                                        iden_sb[:])
                    nc.scalar.activation(ct[:, h, :], pt[:], AF.Relu,
                                         bias=bT_sb[:, h:h + 1],
                                         scale=iavT_sb[:, h:h + 1])
                return ct

            def fin1(t, ups, rden, n, iav_sb, bf_sb):
                ct = head_merge(ups, rden, n, iav_sb, bf_sb)
                ps2 = ps2p.tile([128, 2 * HC], F32, tag="ps2")
                nc.tensor.matmul(ps2[:], ct[:, 0, :], w2_sb[:, 0, :],
                                 start=True, stop=False)
                nc.tensor.matmul(ps2[:], ct[:, 1, :], w2_sb[:, 1, :],
                                 start=False, stop=True)
                l2t = fpool.tile([128, 2, HC], BF16, tag="l2t")
                nc.vector.tensor_copy(l2t[:, 0, :], ps2[:, 0:HC])
                nc.scalar.activation(l2t[:, 1, :], ps2[:, HC:2 * HC], AF.Copy)
                nc.sync.dma_start(out=loc2[t], in_=l2t[:])
                if t < NT_A:
                    nc.scalar.dma_start(out=own2A[t], in_=l2t[:, 0, :])
                else:
                    nc.scalar.dma_start(out=own2B[t - NT_A], in_=l2t[:, 0, :])

            def fin2(t, ups, rden, n, iav_sb, bf_sb):
                ct = head_merge(ups, rden, n, iav_sb, bf_sb)
                zt_ps = psF.tile([128, 128], F32, tag="ztps")
                nc.tensor.matmul(zt_ps[:], w3_sb[:, 0, :], ct[:, 0, :],
                                 start=True, stop=False)
                nc.tensor.matmul(zt_ps[:], w3_sb[:, 1, :], ct[:, 1, :],
                                 start=False, stop=True)
                zt = fpool.tile([128, 128], BF16, tag="zt")
                nc.scalar.activation(zt[:], zt_ps[:], AF.Identity,
                                     bias=b3c_sb[:], scale=1.0)
                o_ps = psF.tile([128, OUT_F], F32, tag="ops")
                nc.tensor.matmul(o_ps[:], zt[:], w4_sb[:], start=True,
                                 stop=True)
                o_pre = fpool.tile([128, OUT_F], F32, tag="opre")
                nc.vector.scalar_tensor_tensor(
                    out=o_pre[:], in0=o_ps[:], scalar=1.0, in1=b4f_sb[:],
                    op0=OP.mult, op1=OP.add)
                o_sb = fpool.tile([128, OUT_F], F32, tag="osb")
                nc.scalar.activation(o_sb[:], o_pre[:], AF.Sigmoid)
                nc.sync.dma_start(out=out_ext[t * 128:(t + 1) * 128, :],
                                  in_=o_sb[:])

            # ================= phase schedule =================
            import os as _os
            _upto = int(_os.environ.get("KPHASES", "9"))
            if _upto >= 2:
                conv(tab1A, tab1B, loc1, LR1, iavT1_sb, bT1_sb, fin1,
                     cc_next=lambda ci: chunk_cc(own2A, own2B, tab2A, tab2B,
                                                 ci) if _upto >= 3 else None)
            if _upto >= 4:
                conv(tab2A, tab2B, loc2, LR2, iavT2_sb, bT2_sb, fin2)
            else:
                zt = fpool.tile([128, OUT_F], F32, tag="osb")
                nc.vector.memset(zt[:], 0.0)
                for t in range(NTILES):
                    nc.sync.dma_start(out=out_ext[t * 128:(t + 1) * 128, :],
                                      in_=zt[:])

    nc.compile()
    return nc


# ---------------------------------------------------------------- entry point
def kernel(**inputs):
    from concourse import bass_utils

    src = np.asarray(inputs["edge_index"][0], np.int64)
    dst = np.asarray(inputs["edge_index"][1], np.int64)
    x = np.asarray(inputs["x"], np.float32)

    pack = _pack_graph(src, dst)
    nos = pack["node_of_slot"]
    valid_slot = nos >= 0
    x_slot = np.zeros((S, IN_F), np.float32)
    x_slot[valid_slot] = x[nos[valid_slot]]

    def bf(a):
        return np.ascontiguousarray(np.asarray(a, np.float32)).astype(BF)

    # per-head column permutation (+att cols first) + pre-scale by att
    def prep_layer(att):
        att = np.asarray(att, np.float32).reshape(2, 128)
        perm = np.zeros(HC, np.int64)
        ranges = []
        for h in range(2):
            a = att[h]
            pos = np.where(a > 0)[0]
            neg = np.where(a <= 0)[0]
            perm[h * 128:(h + 1) * 128] = h * 128 + np.concatenate([pos, neg])
            p = len(pos)
            if p:
                ranges.append((h * 128, h * 128 + p, "max"))
            if p < 128:
                ranges.append((h * 128 + p, (h + 1) * 128, "min"))
        att_p = att.reshape(HC)[perm]
        att_p = np.where(np.abs(att_p) < 1e-30, 1e-30, att_p)
        return perm, att_p, ranges

    perm1, att1p, LR1 = prep_layer(inputs["att1"])
    perm2, att2p, LR2 = prep_layer(inputs["att2"])
    _PLAN["LR1"] = LR1
    _PLAN["LR2"] = LR2

    Wl1p = np.asarray(inputs["Wl1"], np.float32)[:, perm1] * att1p[None, :]
    Wr1p = np.asarray(inputs["Wr1"], np.float32)[:, perm1] * att1p[None, :]
    Wl2p = (np.asarray(inputs["Wl2"], np.float32)[perm1][:, perm2]
            * att2p[None, :])
    Wr2p = (np.asarray(inputs["Wr2"], np.float32)[perm1][:, perm2]
            * att2p[None, :])
    W3p = np.asarray(inputs["W3"], np.float32)[perm2]
    b1p = np.asarray(inputs["b1"], np.float32)[perm1]
    b2p = np.asarray(inputs["b2"], np.float32)[perm2]

    w2c = np.concatenate([Wl2p, Wr2p], 1)           # [256, 512]
    common = {
        "w1cat": bf(np.concatenate([Wl1p, Wr1p], 1)),
        "w2cat": bf(w2c.reshape(2, 128, 2 * HC).transpose(1, 0, 2)),
        "w3": bf(W3p.reshape(2, 128, 128).transpose(1, 0, 2)),
        "w4": bf(inputs["W4"]),
        "iavT1": np.ascontiguousarray(
            (1.0 / att1p).reshape(2, 128).T).astype(np.float32),
        "iavT2": np.ascontiguousarray(
            (1.0 / att2p).reshape(2, 128).T).astype(np.float32),
        "bT1": np.ascontiguousarray(b1p.reshape(2, 128).T).astype(np.float32),
        "bT2": np.ascontiguousarray(b2p.reshape(2, 128).T).astype(np.float32),
        "b3c": np.asarray(inputs["b3"], np.float32).reshape(128, 1),
        "b4f": np.tile(np.asarray(inputs["b4"], np.float32)[None, :],
                       (128, 1)),
        "idenBF": np.eye(128, dtype=np.float32).astype(BF),
    }

    in_maps = []
    for k in range(NCORES):
        m = dict(common)
        m["xoT"] = np.ascontiguousarray(
            x_slot[k * SPC:(k + 1) * SPC].T).astype(BF)
        m["idxA_d"] = pack["idxA_d"][k]
        m["idxB_d"] = pack["idxB_d"][k]
        m["valid_d"] = pack["valid_d"][k]
        in_maps.append(m)

    if "nc" not in _NC_CACHE:
        _NC_CACHE["nc"] = _build_nc()
    nc = _NC_CACHE["nc"]

    res = bass_utils.run_bass_kernel_spmd(nc, in_maps,
                                          core_ids=list(range(NCORES)),
                                          **_RUN_OPTS)
    _LAST_RESULTS["res"] = res
    out_slots = np.concatenate([res.results[k]["out"] for k in range(NCORES)],
                               0)
    return out_slots[pack["slot_of_node"]].astype(np.float32)
